# revision 17
# baseline (speedup 1.0000x reference)
"""EnhancedMultiHeadAttention TRN2 kernel — 8-core query-sharded SPMD.

B=2, S=2048, D=512, H=8, DK=64. Outputs (y, attn):
  attn = softmax(QK^T/8 + tril(exp(-0.02|i-j|)))      (2,8,2048,2048)
  y    = LN(gate-mix(attn@V @ wo + bo, x))            (2,2048,512)

Sharding: core c owns query blocks {c, 15-c} (128 rows each) of BOTH batches
for ALL heads -> no cross-core communication. K/V projections are recomputed
per core over the full sequence. One SPMD program; per-core differences are
input data only (zero-padded bias tiles make the shared block structure
correct for every core).

Pipeline per batch:
  - K^T/V/Q projections per head-pair (bf16 K/V path, f32r Q/gate path)
  - sweep 1: transposed scores [k,q] (head-merged PSUM tiles) -> +bias (DVE
    STT) -> exp (ACT, 2-kblock groups) -> AV with ones-augmented V (rowsum
    free) -> normalized out^T; rowsums transposed to [q,1], reciprocal
  - one batched Ln gives -ln(rowsum) for all heads (ACT table stays on Exp
    during the sweeps)
  - sweep 2: [q,k] scores + bias via identity-matmul accumulation -> single
    fused ACT pass attn = Exp(s - ln(rowsum)) -> DMA out
  - gate MLP, z = out^T.T @ wo, gated residual mix (STT), LayerNorm
    (bn_stats/bn_aggr), DMA y
"""
import numpy as np
from contextlib import ExitStack

import ml_dtypes

import concourse.bass as bass
import concourse.tile as tile
from concourse import bacc, mybir
from concourse.bass_utils import run_bass_kernel_spmd

f32 = mybir.dt.float32
f32r = mybir.dt.float32r
bf16 = mybir.dt.bfloat16
AF = mybir.ActivationFunctionType
OP = mybir.AluOpType
ts = bass.ts

B, S, D, H, DK = 2, 2048, 512, 8, 64
DECAY = 0.02
LN_EPS = 1e-5
NCORES = 8
QB = 128
QPC = 2 * QB      # 256 queries per batch per core
KB = 128
NKB = S // KB     # 16
KS = 512
NKS = S // KS     # 4
NDT = D // 128    # 4
R0_NKS, R1_NKS = 2, 4   # [q,k]-path bias k-slices per block role

_CACHE = {}


def _build(cfg):
    use_bias_q, use_bias_k, use_row_const, use_ln_aff = cfg
    nc = bacc.Bacc("TRN2", target_bir_lowering=False, debug=False,
                   num_devices=NCORES)

    xT_d = nc.dram_tensor("xT", [D, B * S], bf16, kind="ExternalInput").ap()
    xqT_d = nc.dram_tensor("xqT", [D, B * QPC], f32r, kind="ExternalInput").ap()
    xrows_d = nc.dram_tensor("xrows", [2 * B, QB, D], f32, kind="ExternalInput").ap()
    wq8_d = nc.dram_tensor("wq8", [D, D], f32r, kind="ExternalInput").ap()
    wk_d = nc.dram_tensor("wk", [D, D], bf16, kind="ExternalInput").ap()
    wv4_d = nc.dram_tensor("wv4", [D, 2 * 260], bf16, kind="ExternalInput").ap()
    wo_d = nc.dram_tensor("wo", [D, D], f32r, kind="ExternalInput").ap()
    g1w_d = nc.dram_tensor("g1w", [D, 256], f32r, kind="ExternalInput").ap()
    g2w_d = nc.dram_tensor("g2w", [256, 1], f32r, kind="ExternalInput").ap()
    bqk_d = nc.dram_tensor("bias_qk", [2, QB, S], bf16, kind="ExternalInput").ap()
    bT_d = nc.dram_tensor("biasT", [KB, NKB, QPC], bf16, kind="ExternalInput").ap()
    ident_d = nc.dram_tensor("ident", [128, 128], bf16, kind="ExternalInput").ap()
    bq8_d = nc.dram_tensor("bq8", [D, 1], f32, kind="ExternalInput").ap()
    bk_d = nc.dram_tensor("bk", [D, 1], f32, kind="ExternalInput").ap()
    g1b_d = nc.dram_tensor("g1b", [256, 1], f32, kind="ExternalInput").ap()
    rowc_d = nc.dram_tensor("row_const", [1, D], f32, kind="ExternalInput").ap()
    lng_d = nc.dram_tensor("ln_g_row", [1, D], f32, kind="ExternalInput").ap()
    lnb_d = nc.dram_tensor("ln_b_row", [1, D], f32, kind="ExternalInput").ap()
    g2b_d = nc.dram_tensor("g2b", [1, 1], f32, kind="ExternalInput").ap()
    attn_d = nc.dram_tensor("attn_part", [B, H, 2, QB, S], bf16,
                            kind="ExternalOutput").ap()
    y_d = nc.dram_tensor("y_part", [B, 2, QB, D], f32, kind="ExternalOutput").ap()

    def bcast_row(ap_1xN, parts):
        return bass.AP(tensor=ap_1xN.tensor, offset=ap_1xN.offset,
                       ap=[[0, parts]] + list(ap_1xN.ap[1:]))

    with tile.TileContext(nc) as tc, ExitStack() as ctx:
        const = ctx.enter_context(tc.tile_pool(name="const", bufs=1))
        xtp = ctx.enter_context(tc.tile_pool(name="xtp", bufs=4))
        ktp = ctx.enter_context(tc.tile_pool(name="ktp", bufs=4))
        vp = ctx.enter_context(tc.tile_pool(name="vp", bufs=32))
        qtp = ctx.enter_context(tc.tile_pool(name="qtp", bufs=8))
        xqp = ctx.enter_context(tc.tile_pool(name="xqp", bufs=8))
        expp = ctx.enter_context(tc.tile_pool(name="expp", bufs=14))
        sbp = ctx.enter_context(tc.tile_pool(name="sbp", bufs=3))
        attnp = ctx.enter_context(tc.tile_pool(name="attnp", bufs=4))
        outp = ctx.enter_context(tc.tile_pool(name="outp", bufs=8))
        sm = ctx.enter_context(tc.tile_pool(name="sm", bufs=2))
        ps = ctx.enter_context(tc.tile_pool(name="ps", bufs=2, space="PSUM"))

        # ---- constants (wk/wv4 loaded first: needed by first projections) ----
        wq8 = const.tile([128, NDT, D], f32r, name="wq8t")
        wk = const.tile([128, NDT, D], bf16, name="wkt")
        wv4 = const.tile([128, NDT, 520], bf16, name="wv4t")
        wo = const.tile([128, NDT, D], f32r, name="wot")
        g1w = const.tile([128, NDT, 256], f32r, name="g1wt")
        g2w = const.tile([128, 2, 1], f32r, name="g2wt")
        for dt in range(NDT):
            nc.sync.dma_start(wk[:, dt, :], wk_d[ts(dt, 128), :])
        for dt in range(NDT):
            nc.sync.dma_start(wv4[:, dt, :], wv4_d[ts(dt, 128), :])
        for dt in range(NDT):
            nc.gpsimd.dma_start(wq8[:, dt, :], wq8_d[ts(dt, 128), :])
            nc.gpsimd.dma_start(wo[:, dt, :], wo_d[ts(dt, 128), :])
            nc.gpsimd.dma_start(g1w[:, dt, :], g1w_d[ts(dt, 128), :])
        for t in range(2):
            nc.gpsimd.dma_start(g2w[:, t, :], g2w_d[ts(t, 128), :])
        ident = const.tile([128, 128], bf16, name="identt")
        nc.gpsimd.dma_start(ident[:], ident_d[:])
        bqk = const.tile([128, 2, S], bf16, name="bqkt")
        for qt in range(2):
            nc.gpsimd.dma_start(bqk[:, qt, :], bqk_d[qt, :, :])
        bT = const.tile([128, NKB, QPC], bf16, name="bTt")
        nc.gpsimd.dma_start(bT[:], bT_d[:])
        bq8 = const.tile([128, NDT, 1], f32, name="bq8t")
        bk = const.tile([128, NDT, 1], f32, name="bkt")
        for dt in range(NDT):
            nc.gpsimd.dma_start(bq8[:, dt, :], bq8_d[ts(dt, 128), :])
            nc.gpsimd.dma_start(bk[:, dt, :], bk_d[ts(dt, 128), :])
        g1b = const.tile([128, 2, 1], f32, name="g1bt")
        for t in range(2):
            nc.gpsimd.dma_start(g1b[:, t, :], g1b_d[ts(t, 128), :])
        g2b = const.tile([1, 1], f32, name="g2bt")
        nc.gpsimd.dma_start(g2b[:], g2b_d[:])
        one11 = const.tile([1, 1], f32, name="one11")
        nc.vector.memset(one11[:], 1.0)
        epst = const.tile([128, 1], f32, name="epst")
        nc.vector.memset(epst[:], LN_EPS)
        xrows = const.tile([128, 2 * B, D], f32, name="xrowst")
        for i in range(2 * B):
            nc.gpsimd.dma_start(xrows[:, i, :], xrows_d[i, :, :])
        if use_row_const:
            rowc_bc = const.tile([128, D], f32, name="rowcbc")
            nc.gpsimd.dma_start(rowc_bc[:], bcast_row(rowc_d[0:1, :], 128))
        if use_ln_aff:
            lng_bc = const.tile([128, D], f32, name="lngbc")
            lnb_bc = const.tile([128, D], f32, name="lnbbc")
            nc.gpsimd.dma_start(lng_bc[:], bcast_row(lng_d[0:1, :], 128))
            nc.gpsimd.dma_start(lnb_bc[:], bcast_row(lnb_d[0:1, :], 128))

        for b in range(B):
            xT = [xtp.tile([128, S], bf16, name=f"xT_{b}_{dt}", tag="xT")
                  for dt in range(NDT)]
            for dt in range(NDT):
                nc.sync.dma_start(xT[dt][:], xT_d[ts(dt, 128), b * S:(b + 1) * S])
            xq = [xqp.tile([128, QPC], f32r, name=f"xq_{b}_{dt}", tag="xq")
                  for dt in range(NDT)]
            for dt in range(NDT):
                nc.gpsimd.dma_start(xq[dt][:],
                                  xqT_d[ts(dt, 128), b * QPC:(b + 1) * QPC])

            # ---- projections ----
            kT = [ktp.tile([128, S], bf16, name=f"kT_{b}_{j}", tag="kT")
                  for j in range(4)]
            qT = [qtp.tile([128, QPC], bf16, name=f"qT_{b}_{j}", tag="qT")
                  for j in range(4)]
            for j in range(4):
                for sb_i in range(NKS):
                    pk = ps.tile([128, KS], f32, name=f"pk_{b}_{j}_{sb_i}",
                                 tag="proj")
                    for dt in range(NDT):
                        nc.tensor.matmul(pk[:], wk[:, dt, ts(j, 128)],
                                         xT[dt][:, ts(sb_i, KS)],
                                         start=(dt == 0), stop=(dt == NDT - 1))
                    if use_bias_k:
                        nc.vector.tensor_scalar_add(kT[j][:, ts(sb_i, KS)],
                                                    pk[:], bk[:, j, :])
                    else:
                        nc.vector.tensor_copy(kT[j][:, ts(sb_i, KS)], pk[:])
                pq = ps.tile([128, QPC], f32, name=f"pq_{b}_{j}", tag="proj")
                for dt in range(NDT):
                    nc.tensor.matmul(pq[:], wq8[:, dt, ts(j, 128)], xq[dt][:],
                                     start=(dt == 0), stop=(dt == NDT - 1))
                if use_bias_q:
                    nc.vector.tensor_scalar_add(qT[j][:], pq[:], bq8[:, j, :])
                else:
                    nc.vector.tensor_copy(qT[j][:], pq[:])
            vt = {}
            for q2 in range(2):
                for st in range(NKB):
                    v_t = vp.tile([128, 260], bf16, name=f"v_{b}_{q2}_{st}",
                                  tag="v")
                    pv = ps.tile([128, 260], f32, name=f"pv_{b}_{q2}_{st}",
                                 tag="proj")
                    for dt in range(NDT):
                        nc.tensor.matmul(pv[:], xT[dt][:, ts(st, 128)],
                                         wv4[:, dt, ts(q2, 260)],
                                         start=(dt == 0), stop=(dt == NDT - 1))
                    nc.scalar.copy(v_t[:], pv[:])
                    vv = v_t.rearrange("p (a c) -> p a c", c=65)
                    nc.gpsimd.memset(vv[:, :, 64:65], 1.0)
                    vt[(q2, st)] = v_t

            # ---- sweeps in two half-batches (4 heads each) so the [q,k]
            # sweep of one half overlaps the transposed sweep of the next ----
            outT = [outp.tile([128, QPC], f32r, name=f"outT_{b}_{dt2}",
                              tag="outT") for dt2 in range(NDT)]
            for half in range(2):
                rq = sm.tile([128, 4, 2], f32, name=f"rq_{b}_{half}",
                             tag="rq", bufs=3)
                for j in (2 * half, 2 * half + 1):
                    etg = []
                    for g in range(NKB // 2):
                        sbg = sbp.tile([128, 2, 2, QPC], f32,
                                       name=f"sbg_{b}_{j}_{g}", tag="sbg")
                        eg = expp.tile([128, 2, 2, QPC], bf16,
                                       name=f"eg_{b}_{j}_{g}", tag="eg")
                        for r in range(2):
                            pst = ps.tile([128, 2, QPC], f32,
                                          name=f"pst_{b}_{j}_{g}_{r}",
                                          tag="sT", bufs=3)
                            for kk in range(2):
                                kb = 2 * g + kk
                                nc.tensor.matmul(pst[:, kk],
                                                 kT[j][ts(r, 64), ts(kb, KB)],
                                                 qT[j][ts(r, 64), :],
                                                 start=True, stop=True)
                            nc.vector.scalar_tensor_tensor(
                                sbg[:, :, r, :], pst[:], 1.0,
                                bT[:, 2 * g:2 * g + 2, :],
                                op0=OP.mult, op1=OP.add)
                        nc.scalar.activation(eg[:], sbg[:], AF.Exp)
                        etg.append(eg)
                    for r in range(2):
                        h = 2 * j + r
                        pav = ps.tile([65, QPC], f32, name=f"pav_{b}_{h}",
                                      tag="av", bufs=1)
                        for kb in range(NKB):
                            nc.tensor.matmul(pav[:],
                                             vt[(j // 2, kb)][:, ts(h % 4, 65)],
                                             etg[kb // 2][:, kb % 2, r],
                                             start=(kb == 0),
                                             stop=(kb == NKB - 1))
                        rs_row = sm.tile([1, QPC], f32, name=f"rs_{b}_{h}",
                                         tag="rs", bufs=3)
                        nc.vector.tensor_copy(rs_row[:], pav[64:65, :])
                        nc.vector.tensor_copy(outT[j][ts(r, 64), :],
                                              pav[0:64, :])
                        rbc = sm.tile([128, QPC], f32, name=f"rbc_{b}_{h}",
                                      tag="rbc", bufs=2)
                        nc.gpsimd.partition_broadcast(rbc[:], rs_row[:])
                        nc.vector.reciprocal(rbc[ts(r, 64), :],
                                             rbc[ts(r, 64), :])
                        nc.vector.tensor_mul(outT[j][ts(r, 64), :],
                                             outT[j][ts(r, 64), :],
                                             rbc[ts(r, 64), :])
                        for qt in range(2):
                            ptr = ps.tile([128, 1], f32,
                                          name=f"ptr_{b}_{h}_{qt}", tag="proj")
                            nc.tensor.matmul(ptr[:], rs_row[0:1, ts(qt, QB)],
                                             one11[:], start=True, stop=True)
                            nc.vector.reciprocal(rq[:, h - 4 * half, qt:qt + 1],
                                                 ptr[:])
                negln = sm.tile([128, 4, 2], f32, name=f"negln_{b}_{half}",
                                tag="negln", bufs=3)
                nc.scalar.activation(negln[:], rq[:], AF.Ln)

                for j in (2 * half, 2 * half + 1):
                    for r in range(2):
                        h = 2 * j + r
                        for qt in range(2):
                            at = attnp.tile([128, S], bf16,
                                            name=f"at_{b}_{h}_{qt}", tag="attn")
                            nks_bias = R0_NKS if qt == 0 else R1_NKS
                            for sl in range(NKS):
                                psl = ps.tile([128, KS], f32,
                                              name=f"psl_{b}_{h}_{qt}_{sl}",
                                              tag="sqk")
                                has_bias = sl < nks_bias
                                nc.tensor.matmul(psl[:],
                                                 qT[j][ts(r, 64), ts(qt, QB)],
                                                 kT[j][ts(r, 64), ts(sl, KS)],
                                                 start=True, stop=not has_bias)
                                if has_bias:
                                    nc.tensor.matmul(psl[:], ident[:],
                                                     bqk[:, qt, ts(sl, KS)],
                                                     start=False, stop=True)
                                nc.scalar.activation(
                                    at[:, ts(sl, KS)], psl[:], AF.Exp,
                                    bias=negln[:, h - 4 * half, qt:qt + 1])
                            nc.gpsimd.dma_start(attn_d[b, h, qt], at[:])

            # ---- gate MLP ----
            reluT = [sm.tile([128, QPC], f32r, name=f"relu_{b}_{t}",
                             tag="reluT", bufs=2) for t in range(2)]
            for t in range(2):
                pg = ps.tile([128, QPC], f32, name=f"pg_{b}_{t}", tag="proj")
                for dt in range(NDT):
                    nc.tensor.matmul(pg[:], g1w[:, dt, ts(t, 128)], xq[dt][:],
                                     start=(dt == 0), stop=(dt == NDT - 1))
                nc.scalar.activation(reluT[t][:], pg[:], AF.Relu,
                                     bias=g1b[:, t, :])
            pgp = ps.tile([1, QPC], f32, name=f"pgp_{b}", tag="proj")
            for t in range(2):
                nc.tensor.matmul(pgp[:], g2w[:, t, :], reluT[t][:],
                                 start=(t == 0), stop=(t == 1))
            gsig = sm.tile([1, QPC], f32, name=f"gsig_{b}", tag="gsig")
            nc.scalar.activation(gsig[:], pgp[:], AF.Sigmoid, bias=g2b[0:1, :])
            gate = sm.tile([128, 2, 1], f32, name=f"gate_{b}", tag="gate")
            for qt in range(2):
                pt = ps.tile([128, 1], f32, name=f"ptg_{b}_{qt}", tag="proj")
                nc.tensor.matmul(pt[:], gsig[0:1, ts(qt, QB)], one11[:],
                                 start=True, stop=True)
                nc.vector.tensor_copy(gate[:, qt, :], pt[:])

            # ---- z, gated residual, LayerNorm ----
            for qt in range(2):
                pz = ps.tile([128, D], f32, name=f"pz_{b}_{qt}", tag="sqk")
                for dt in range(NDT):
                    nc.tensor.matmul(pz[:], outT[dt][:, ts(qt, QB)],
                                     wo[:, dt, :], start=(dt == 0),
                                     stop=(dt == NDT - 1))
                z = sm.tile([128, D], f32, name=f"z_{b}_{qt}", tag="z")
                nc.vector.tensor_copy(z[:], pz[:])
                if use_row_const:
                    nc.vector.tensor_add(z[:], z[:], rowc_bc[:])
                xr = xrows[:, 2 * b + qt, :]
                dlt = sm.tile([128, D], f32, name=f"d_{b}_{qt}", tag="dlt")
                nc.vector.tensor_sub(dlt[:], z[:], xr)
                pre = sm.tile([128, D], f32, name=f"pre_{b}_{qt}", tag="pre",
                              bufs=1)
                nc.vector.scalar_tensor_tensor(pre[:], dlt[:], gate[:, qt, :],
                                               xr, op0=OP.mult, op1=OP.add)
                stats = sm.tile([128, 6], f32, name=f"st_{b}_{qt}", tag="stats")
                nc.vector.bn_stats(stats[:], pre[:])
                mv = sm.tile([128, 2], f32, name=f"mv_{b}_{qt}", tag="mv")
                nc.vector.bn_aggr(mv[:], stats[:])
                sd = sm.tile([128, 1], f32, name=f"sd_{b}_{qt}", tag="sd")
                nc.scalar.activation(sd[:], mv[:, 1:2], AF.Sqrt, bias=epst[:])
                rstd = sm.tile([128, 1], f32, name=f"rstd_{b}_{qt}", tag="rstd")
                nc.vector.reciprocal(rstd[:], sd[:])
                nb = sm.tile([128, 1], f32, name=f"nb_{b}_{qt}", tag="nb")
                nc.vector.tensor_mul(nb[:], mv[:, 0:1], rstd[:])
                nc.vector.tensor_scalar_mul(nb[:], nb[:], -1.0)
                yt = sm.tile([128, D], f32, name=f"y_{b}_{qt}", tag="y",
                             bufs=2)
                nc.scalar.activation(yt[:], pre[:], AF.Identity, bias=nb[:],
                                     scale=rstd[:])
                if use_ln_aff:
                    nc.vector.tensor_mul(yt[:], yt[:], lng_bc[:])
                    nc.vector.tensor_add(yt[:], yt[:], lnb_bc[:])
                nc.gpsimd.dma_start(y_d[b, qt], yt[:])

    nc.compile()
    return nc


def kernel(**inputs):
    x = np.ascontiguousarray(np.asarray(inputs["x"], dtype=np.float32))
    wq = np.asarray(inputs["wq"], np.float32); bq = np.asarray(inputs["bq"], np.float32)
    wk = np.asarray(inputs["wk"], np.float32); bk = np.asarray(inputs["bk"], np.float32)
    wv = np.asarray(inputs["wv"], np.float32); bv = np.asarray(inputs["bv"], np.float32)
    wo = np.asarray(inputs["wo"], np.float32); bo = np.asarray(inputs["bo"], np.float32)
    g1w = np.asarray(inputs["g1_w"], np.float32); g1b = np.asarray(inputs["g1_b"], np.float32)
    g2w = np.asarray(inputs["g2_w"], np.float32); g2b = np.asarray(inputs["g2_b"], np.float32)
    lng = np.asarray(inputs["ln_g"], np.float32); lnb = np.asarray(inputs["ln_b"], np.float32)

    use_bias_q = bool(np.any(bq)); use_bias_k = bool(np.any(bk))
    row_const = bv @ wo + bo
    use_row_const = bool(np.any(row_const))
    use_ln_aff = bool(np.any(lnb)) or not np.allclose(lng, 1.0)
    cfg = (use_bias_q, use_bias_k, use_row_const, use_ln_aff)
    if cfg not in _CACHE:
        _CACHE[cfg] = _build(cfg)
    nc = _CACHE[cfg]

    bf = ml_dtypes.bfloat16
    xT = np.ascontiguousarray(np.transpose(x, (2, 0, 1)).reshape(D, B * S))
    wv4 = np.zeros((D, 2 * 260), np.float32)
    for h in range(H):
        q2, m = divmod(h, 4)
        wv4[:, 260 * q2 + 65 * m: 260 * q2 + 65 * m + 64] = \
            wv[:, 64 * h:64 * h + 64]
    pos = np.arange(S, dtype=np.float64)

    shared = {
        "xT": xT.astype(bf), "wq8": np.ascontiguousarray(wq * 0.125),
        "wk": wk.astype(bf), "wv4": wv4.astype(bf), "wo": wo, "g1w": g1w,
        "g2w": g2w, "ident": np.eye(128, dtype=np.float32).astype(bf),
        "bq8": np.ascontiguousarray((bq * 0.125).reshape(D, 1)),
        "bk": np.ascontiguousarray(bk.reshape(D, 1)),
        "g1b": np.ascontiguousarray(g1b.reshape(256, 1)),
        "row_const": np.ascontiguousarray(row_const.reshape(1, D)),
        "ln_g_row": np.ascontiguousarray(lng.reshape(1, D)),
        "ln_b_row": np.ascontiguousarray(lnb.reshape(1, D)),
        "g2b": np.asarray(g2b, np.float32).reshape(1, 1),
    }
    in_maps = []
    for c in range(NCORES):
        qbs = [c, 15 - c]
        qidx = np.concatenate([np.arange(QB * qb, QB * qb + QB) for qb in qbs])
        bias_full = np.zeros((2, QB, S), np.float64)
        for qt, qb in enumerate(qbs):
            qv = pos[QB * qb: QB * qb + QB][:, None]
            kv = pos[None, :]
            bias_full[qt] = np.exp(-DECAY * np.abs(qv - kv)) * (qv >= kv)
        biasT = np.zeros((KB, NKB, QPC), np.float64)
        qv = pos[qidx][None, :]
        for kb_i in range(NKB):
            kv = pos[KB * kb_i: KB * kb_i + KB][:, None]
            biasT[:, kb_i, :] = np.exp(-DECAY * np.abs(kv - qv)) * (qv >= kv)
        xq_rows = x[:, qidx, :]
        xrows = np.ascontiguousarray(xq_rows.reshape(2 * B, QB, D))
        xqT = np.ascontiguousarray(
            np.transpose(xq_rows, (2, 0, 1)).reshape(D, B * QPC))
        m = dict(shared)
        m["xrows"] = xrows
        m["xqT"] = xqT
        m["bias_qk"] = bias_full.astype(bf)
        m["biasT"] = biasT.astype(bf)
        in_maps.append(m)

    res = run_bass_kernel_spmd(nc, in_maps, core_ids=list(range(NCORES)))

    attn = np.empty((B, H, S, S), np.float32)
    y = np.empty((B, S, D), np.float32)
    for c in range(NCORES):
        ap = np.asarray(res.results[c]["attn_part"], dtype=np.float32)
        yp = res.results[c]["y_part"]
        for qt, qb in enumerate([c, 15 - c]):
            attn[:, :, QB * qb: QB * qb + QB, :] = ap[:, :, qt]
            y[:, QB * qb: QB * qb + QB, :] = yp[:, qt]
    return y, attn


# revision 19
# speedup vs baseline: 1.0699x; 1.0699x over previous
"""EnhancedMultiHeadAttention TRN2 kernel — 8-core query-sharded SPMD.

B=2, S=2048, D=512, H=8, DK=64. Outputs (y, attn):
  attn = softmax(QK^T/8 + tril(exp(-0.02|i-j|)))      (2,8,2048,2048)
  y    = LN(gate-mix(attn@V @ wo + bo, x))            (2,2048,512)

Sharding: core c owns query blocks {c, 15-c} (128 rows each) of BOTH batches
for ALL heads -> no cross-core communication. K/V projections are recomputed
per core over the full sequence. One SPMD program; per-core differences are
input data only (zero-padded bias tiles make the shared block structure
correct for every core).

Pipeline per batch:
  - K^T/V/Q projections per head-pair (bf16 K/V path, f32r Q/gate path)
  - sweep 1: transposed scores [k,q] (head-merged PSUM tiles) -> +bias (DVE
    STT) -> exp (ACT, 2-kblock groups) -> AV with ones-augmented V (rowsum
    free) -> normalized out^T; rowsums transposed to [q,1], reciprocal
  - one batched Ln gives -ln(rowsum) for all heads (ACT table stays on Exp
    during the sweeps)
  - sweep 2: [q,k] scores + bias via identity-matmul accumulation -> single
    fused ACT pass attn = Exp(s - ln(rowsum)) -> DMA out
  - gate MLP, z = out^T.T @ wo, gated residual mix (STT), LayerNorm
    (bn_stats/bn_aggr), DMA y
"""
import numpy as np
from contextlib import ExitStack

import ml_dtypes

import concourse.bass as bass
import concourse.tile as tile
from concourse import bacc, mybir
from concourse.bass_utils import run_bass_kernel_spmd

f32 = mybir.dt.float32
f32r = mybir.dt.float32r
bf16 = mybir.dt.bfloat16
AF = mybir.ActivationFunctionType
OP = mybir.AluOpType
ts = bass.ts

B, S, D, H, DK = 2, 2048, 512, 8, 64
DECAY = 0.02
LN_EPS = 1e-5
NCORES = 8
QB = 128
QPC = 2 * QB      # 256 queries per batch per core
KB = 128
NKB = S // KB     # 16
KS = 512
NKS = S // KS     # 4
NDT = D // 128    # 4
R0_NKS, R1_NKS = 2, 4   # [q,k]-path bias k-slices per block role

_CACHE = {}


def _build(cfg):
    use_bias_q, use_bias_k, use_row_const, use_ln_aff = cfg
    nc = bacc.Bacc("TRN2", target_bir_lowering=False, debug=False,
                   num_devices=NCORES)

    xT_d = nc.dram_tensor("xT", [D, B * S], bf16, kind="ExternalInput").ap()
    xqT_d = nc.dram_tensor("xqT", [D, B * QPC], f32r, kind="ExternalInput").ap()
    xrows_d = nc.dram_tensor("xrows", [2 * B, QB, D], f32, kind="ExternalInput").ap()
    wq8_d = nc.dram_tensor("wq8", [D, D], f32r, kind="ExternalInput").ap()
    wk_d = nc.dram_tensor("wk", [D, D], bf16, kind="ExternalInput").ap()
    wv4_d = nc.dram_tensor("wv4", [D, 2 * 260], bf16, kind="ExternalInput").ap()
    wo_d = nc.dram_tensor("wo", [D, D], f32r, kind="ExternalInput").ap()
    g1w_d = nc.dram_tensor("g1w", [D, 256], f32r, kind="ExternalInput").ap()
    g2w_d = nc.dram_tensor("g2w", [256, 1], f32r, kind="ExternalInput").ap()
    bqk_d = nc.dram_tensor("bias_qk", [2, QB, S], bf16, kind="ExternalInput").ap()
    bT_d = nc.dram_tensor("biasT", [KB, NKB, QPC], bf16, kind="ExternalInput").ap()
    ident_d = nc.dram_tensor("ident", [128, 128], bf16, kind="ExternalInput").ap()
    bq8_d = nc.dram_tensor("bq8", [D, 1], f32, kind="ExternalInput").ap()
    bk_d = nc.dram_tensor("bk", [D, 1], f32, kind="ExternalInput").ap()
    g1b_d = nc.dram_tensor("g1b", [256, 1], f32, kind="ExternalInput").ap()
    rowc_d = nc.dram_tensor("row_const", [1, D], f32, kind="ExternalInput").ap()
    lng_d = nc.dram_tensor("ln_g_row", [1, D], f32, kind="ExternalInput").ap()
    lnb_d = nc.dram_tensor("ln_b_row", [1, D], f32, kind="ExternalInput").ap()
    g2b_d = nc.dram_tensor("g2b", [1, 1], f32, kind="ExternalInput").ap()
    attn_d = nc.dram_tensor("attn_part", [B, H, 2, QB, S], bf16,
                            kind="ExternalOutput").ap()
    y_d = nc.dram_tensor("y_part", [B, 2, QB, D], f32, kind="ExternalOutput").ap()

    def bcast_row(ap_1xN, parts):
        return bass.AP(tensor=ap_1xN.tensor, offset=ap_1xN.offset,
                       ap=[[0, parts]] + list(ap_1xN.ap[1:]))

    with tile.TileContext(nc) as tc, ExitStack() as ctx:
        const = ctx.enter_context(tc.tile_pool(name="const", bufs=1))
        xtp = ctx.enter_context(tc.tile_pool(name="xtp", bufs=4))
        ktp = ctx.enter_context(tc.tile_pool(name="ktp", bufs=4))
        vp = ctx.enter_context(tc.tile_pool(name="vp", bufs=32))
        qtp = ctx.enter_context(tc.tile_pool(name="qtp", bufs=8))
        xqp = ctx.enter_context(tc.tile_pool(name="xqp", bufs=8))
        expp = ctx.enter_context(tc.tile_pool(name="expp", bufs=14))
        sbp = ctx.enter_context(tc.tile_pool(name="sbp", bufs=3))
        attnp = ctx.enter_context(tc.tile_pool(name="attnp", bufs=4))
        outp = ctx.enter_context(tc.tile_pool(name="outp", bufs=8))
        sm = ctx.enter_context(tc.tile_pool(name="sm", bufs=2))
        ps = ctx.enter_context(tc.tile_pool(name="ps", bufs=2, space="PSUM"))

        # ---- constants (wk/wv4 loaded first: needed by first projections) ----
        wq8 = const.tile([128, NDT, D], f32r, name="wq8t")
        wk = const.tile([128, NDT, D], bf16, name="wkt")
        wv4 = const.tile([128, NDT, 520], bf16, name="wv4t")
        wo = const.tile([128, NDT, D], f32r, name="wot")
        g1w = const.tile([128, NDT, 256], f32r, name="g1wt")
        g2w = const.tile([128, 2, 1], f32r, name="g2wt")
        for dt in range(NDT):
            nc.sync.dma_start(wk[:, dt, :], wk_d[ts(dt, 128), :])
        for dt in range(NDT):
            nc.sync.dma_start(wv4[:, dt, :], wv4_d[ts(dt, 128), :])
        for dt in range(NDT):
            nc.gpsimd.dma_start(wq8[:, dt, :], wq8_d[ts(dt, 128), :])
            nc.gpsimd.dma_start(wo[:, dt, :], wo_d[ts(dt, 128), :])
            nc.gpsimd.dma_start(g1w[:, dt, :], g1w_d[ts(dt, 128), :])
        for t in range(2):
            nc.gpsimd.dma_start(g2w[:, t, :], g2w_d[ts(t, 128), :])
        ident = const.tile([128, 128], bf16, name="identt")
        nc.gpsimd.dma_start(ident[:], ident_d[:])
        bqk = const.tile([128, 2, S], bf16, name="bqkt")
        for qt in range(2):
            nc.gpsimd.dma_start(bqk[:, qt, :], bqk_d[qt, :, :])
        bT = const.tile([128, NKB, QPC], bf16, name="bTt")
        nc.gpsimd.dma_start(bT[:], bT_d[:])
        bq8 = const.tile([128, NDT, 1], f32, name="bq8t")
        bk = const.tile([128, NDT, 1], f32, name="bkt")
        for dt in range(NDT):
            nc.gpsimd.dma_start(bq8[:, dt, :], bq8_d[ts(dt, 128), :])
            nc.gpsimd.dma_start(bk[:, dt, :], bk_d[ts(dt, 128), :])
        g1b = const.tile([128, 2, 1], f32, name="g1bt")
        for t in range(2):
            nc.gpsimd.dma_start(g1b[:, t, :], g1b_d[ts(t, 128), :])
        g2b = const.tile([1, 1], f32, name="g2bt")
        nc.gpsimd.dma_start(g2b[:], g2b_d[:])
        one11 = const.tile([1, 1], f32, name="one11")
        nc.vector.memset(one11[:], 1.0)
        epst = const.tile([128, 1], f32, name="epst")
        nc.vector.memset(epst[:], LN_EPS)
        xrows = const.tile([128, 2 * B, D], f32, name="xrowst")
        for i in range(2 * B):
            nc.gpsimd.dma_start(xrows[:, i, :], xrows_d[i, :, :])
        if use_row_const:
            rowc_bc = const.tile([128, D], f32, name="rowcbc")
            nc.gpsimd.dma_start(rowc_bc[:], bcast_row(rowc_d[0:1, :], 128))
        if use_ln_aff:
            lng_bc = const.tile([128, D], f32, name="lngbc")
            lnb_bc = const.tile([128, D], f32, name="lnbbc")
            nc.gpsimd.dma_start(lng_bc[:], bcast_row(lng_d[0:1, :], 128))
            nc.gpsimd.dma_start(lnb_bc[:], bcast_row(lnb_d[0:1, :], 128))

        for b in range(B):
            xT = [xtp.tile([128, S], bf16, name=f"xT_{b}_{dt}", tag="xT")
                  for dt in range(NDT)]
            for dt in range(NDT):
                nc.scalar.dma_start(xT[dt][:], xT_d[ts(dt, 128), b * S:(b + 1) * S])
            xq = [xqp.tile([128, QPC], f32r, name=f"xq_{b}_{dt}", tag="xq")
                  for dt in range(NDT)]
            for dt in range(NDT):
                nc.gpsimd.dma_start(xq[dt][:],
                                  xqT_d[ts(dt, 128), b * QPC:(b + 1) * QPC])

            # ---- projections ----
            kT = [ktp.tile([128, S], bf16, name=f"kT_{b}_{j}", tag="kT")
                  for j in range(4)]
            qT = [qtp.tile([128, QPC], bf16, name=f"qT_{b}_{j}", tag="qT")
                  for j in range(4)]
            for j in range(4):
                for sb_i in range(NKS):
                    pk = ps.tile([128, KS], f32, name=f"pk_{b}_{j}_{sb_i}",
                                 tag="proj")
                    for dt in range(NDT):
                        nc.tensor.matmul(pk[:], wk[:, dt, ts(j, 128)],
                                         xT[dt][:, ts(sb_i, KS)],
                                         start=(dt == 0), stop=(dt == NDT - 1))
                    if use_bias_k:
                        nc.vector.tensor_scalar_add(kT[j][:, ts(sb_i, KS)],
                                                    pk[:], bk[:, j, :])
                    else:
                        nc.vector.tensor_copy(kT[j][:, ts(sb_i, KS)], pk[:])
                pq = ps.tile([128, QPC], f32, name=f"pq_{b}_{j}", tag="proj")
                for dt in range(NDT):
                    nc.tensor.matmul(pq[:], wq8[:, dt, ts(j, 128)], xq[dt][:],
                                     start=(dt == 0), stop=(dt == NDT - 1))
                if use_bias_q:
                    nc.vector.tensor_scalar_add(qT[j][:], pq[:], bq8[:, j, :])
                else:
                    nc.vector.tensor_copy(qT[j][:], pq[:])
            vt = {}
            for q2 in range(2):
                for st in range(NKB):
                    v_t = vp.tile([128, 260], bf16, name=f"v_{b}_{q2}_{st}",
                                  tag="v")
                    pv = ps.tile([128, 260], f32, name=f"pv_{b}_{q2}_{st}",
                                 tag="proj")
                    for dt in range(NDT):
                        nc.tensor.matmul(pv[:], xT[dt][:, ts(st, 128)],
                                         wv4[:, dt, ts(q2, 260)],
                                         start=(dt == 0), stop=(dt == NDT - 1))
                    nc.scalar.copy(v_t[:], pv[:])
                    vv = v_t.rearrange("p (a c) -> p a c", c=65)
                    nc.gpsimd.memset(vv[:, :, 64:65], 1.0)
                    vt[(q2, st)] = v_t

            # ---- sweeps in two half-batches (4 heads each) so the [q,k]
            # sweep of one half overlaps the transposed sweep of the next ----
            outT = [outp.tile([128, QPC], f32r, name=f"outT_{b}_{dt2}",
                              tag="outT") for dt2 in range(NDT)]
            for half in range(2):
                rq = sm.tile([128, 4, 2], f32, name=f"rq_{b}_{half}",
                             tag="rq", bufs=3)
                for j in (2 * half, 2 * half + 1):
                    etg = []
                    for g in range(NKB // 2):
                        sbg = sbp.tile([128, 2, 2, QPC], f32,
                                       name=f"sbg_{b}_{j}_{g}", tag="sbg")
                        eg = expp.tile([128, 2, 2, QPC], bf16,
                                       name=f"eg_{b}_{j}_{g}", tag="eg")
                        for r in range(2):
                            pst = ps.tile([128, 2, QPC], f32,
                                          name=f"pst_{b}_{j}_{g}_{r}",
                                          tag="sT", bufs=3)
                            for kk in range(2):
                                kb = 2 * g + kk
                                nc.tensor.matmul(pst[:, kk],
                                                 kT[j][ts(r, 64), ts(kb, KB)],
                                                 qT[j][ts(r, 64), :],
                                                 start=True, stop=True)
                            nc.vector.scalar_tensor_tensor(
                                sbg[:, :, r, :], pst[:], 1.0,
                                bT[:, 2 * g:2 * g + 2, :],
                                op0=OP.mult, op1=OP.add)
                        nc.scalar.activation(eg[:], sbg[:], AF.Exp)
                        etg.append(eg)
                    for r in range(2):
                        h = 2 * j + r
                        pav = ps.tile([65, QPC], f32, name=f"pav_{b}_{h}",
                                      tag="av", bufs=1)
                        for kb in range(NKB):
                            nc.tensor.matmul(pav[:],
                                             vt[(j // 2, kb)][:, ts(h % 4, 65)],
                                             etg[kb // 2][:, kb % 2, r],
                                             start=(kb == 0),
                                             stop=(kb == NKB - 1))
                        rs_row = sm.tile([1, QPC], f32, name=f"rs_{b}_{h}",
                                         tag="rs", bufs=3)
                        nc.vector.tensor_copy(rs_row[:], pav[64:65, :])
                        nc.vector.tensor_copy(outT[j][ts(r, 64), :],
                                              pav[0:64, :])
                        rbc = sm.tile([128, QPC], f32, name=f"rbc_{b}_{h}",
                                      tag="rbc", bufs=2)
                        nc.gpsimd.partition_broadcast(rbc[:], rs_row[:])
                        nc.vector.reciprocal(rbc[ts(r, 64), :],
                                             rbc[ts(r, 64), :])
                        nc.vector.tensor_mul(outT[j][ts(r, 64), :],
                                             outT[j][ts(r, 64), :],
                                             rbc[ts(r, 64), :])
                        for qt in range(2):
                            ptr = ps.tile([128, 1], f32,
                                          name=f"ptr_{b}_{h}_{qt}", tag="proj")
                            nc.tensor.matmul(ptr[:], rs_row[0:1, ts(qt, QB)],
                                             one11[:], start=True, stop=True)
                            nc.vector.reciprocal(rq[:, h - 4 * half, qt:qt + 1],
                                                 ptr[:])
                negln = sm.tile([128, 4, 2], f32, name=f"negln_{b}_{half}",
                                tag="negln", bufs=3)
                nc.scalar.activation(negln[:], rq[:], AF.Ln)

                for j in (2 * half, 2 * half + 1):
                    for r in range(2):
                        h = 2 * j + r
                        for qt in range(2):
                            at = attnp.tile([128, S], bf16,
                                            name=f"at_{b}_{h}_{qt}", tag="attn")
                            nks_bias = R0_NKS if qt == 0 else R1_NKS
                            for sl in range(NKS):
                                psl = ps.tile([128, KS], f32,
                                              name=f"psl_{b}_{h}_{qt}_{sl}",
                                              tag="sqk")
                                has_bias = sl < nks_bias
                                nc.tensor.matmul(psl[:],
                                                 qT[j][ts(r, 64), ts(qt, QB)],
                                                 kT[j][ts(r, 64), ts(sl, KS)],
                                                 start=True, stop=not has_bias)
                                if has_bias:
                                    nc.tensor.matmul(psl[:], ident[:],
                                                     bqk[:, qt, ts(sl, KS)],
                                                     start=False, stop=True)
                                nc.scalar.activation(
                                    at[:, ts(sl, KS)], psl[:], AF.Exp,
                                    bias=negln[:, h - 4 * half, qt:qt + 1])
                            nc.gpsimd.dma_start(attn_d[b, h, qt], at[:])

            # ---- gate MLP ----
            reluT = [sm.tile([128, QPC], f32r, name=f"relu_{b}_{t}",
                             tag="reluT", bufs=2) for t in range(2)]
            for t in range(2):
                pg = ps.tile([128, QPC], f32, name=f"pg_{b}_{t}", tag="proj")
                for dt in range(NDT):
                    nc.tensor.matmul(pg[:], g1w[:, dt, ts(t, 128)], xq[dt][:],
                                     start=(dt == 0), stop=(dt == NDT - 1))
                nc.scalar.activation(reluT[t][:], pg[:], AF.Relu,
                                     bias=g1b[:, t, :])
            pgp = ps.tile([1, QPC], f32, name=f"pgp_{b}", tag="proj")
            for t in range(2):
                nc.tensor.matmul(pgp[:], g2w[:, t, :], reluT[t][:],
                                 start=(t == 0), stop=(t == 1))
            gsig = sm.tile([1, QPC], f32, name=f"gsig_{b}", tag="gsig")
            nc.scalar.activation(gsig[:], pgp[:], AF.Sigmoid, bias=g2b[0:1, :])
            gate = sm.tile([128, 2, 1], f32, name=f"gate_{b}", tag="gate")
            for qt in range(2):
                pt = ps.tile([128, 1], f32, name=f"ptg_{b}_{qt}", tag="proj")
                nc.tensor.matmul(pt[:], gsig[0:1, ts(qt, QB)], one11[:],
                                 start=True, stop=True)
                nc.vector.tensor_copy(gate[:, qt, :], pt[:])

            # ---- z, gated residual, LayerNorm ----
            for qt in range(2):
                pz = ps.tile([128, D], f32, name=f"pz_{b}_{qt}", tag="sqk")
                for dt in range(NDT):
                    nc.tensor.matmul(pz[:], outT[dt][:, ts(qt, QB)],
                                     wo[:, dt, :], start=(dt == 0),
                                     stop=(dt == NDT - 1))
                z = sm.tile([128, D], f32, name=f"z_{b}_{qt}", tag="z")
                nc.vector.tensor_copy(z[:], pz[:])
                if use_row_const:
                    nc.vector.tensor_add(z[:], z[:], rowc_bc[:])
                xr = xrows[:, 2 * b + qt, :]
                dlt = sm.tile([128, D], f32, name=f"d_{b}_{qt}", tag="dlt")
                nc.vector.tensor_sub(dlt[:], z[:], xr)
                pre = sm.tile([128, D], f32, name=f"pre_{b}_{qt}", tag="pre",
                              bufs=1)
                nc.vector.scalar_tensor_tensor(pre[:], dlt[:], gate[:, qt, :],
                                               xr, op0=OP.mult, op1=OP.add)
                stats = sm.tile([128, 6], f32, name=f"st_{b}_{qt}", tag="stats")
                nc.vector.bn_stats(stats[:], pre[:])
                mv = sm.tile([128, 2], f32, name=f"mv_{b}_{qt}", tag="mv")
                nc.vector.bn_aggr(mv[:], stats[:])
                sd = sm.tile([128, 1], f32, name=f"sd_{b}_{qt}", tag="sd")
                nc.scalar.activation(sd[:], mv[:, 1:2], AF.Sqrt, bias=epst[:])
                rstd = sm.tile([128, 1], f32, name=f"rstd_{b}_{qt}", tag="rstd")
                nc.vector.reciprocal(rstd[:], sd[:])
                nb = sm.tile([128, 1], f32, name=f"nb_{b}_{qt}", tag="nb")
                nc.vector.tensor_mul(nb[:], mv[:, 0:1], rstd[:])
                nc.vector.tensor_scalar_mul(nb[:], nb[:], -1.0)
                yt = sm.tile([128, D], f32, name=f"y_{b}_{qt}", tag="y",
                             bufs=2)
                nc.scalar.activation(yt[:], pre[:], AF.Identity, bias=nb[:],
                                     scale=rstd[:])
                if use_ln_aff:
                    nc.vector.tensor_mul(yt[:], yt[:], lng_bc[:])
                    nc.vector.tensor_add(yt[:], yt[:], lnb_bc[:])
                nc.gpsimd.dma_start(y_d[b, qt], yt[:])

    nc.compile()
    return nc


def kernel(**inputs):
    x = np.ascontiguousarray(np.asarray(inputs["x"], dtype=np.float32))
    wq = np.asarray(inputs["wq"], np.float32); bq = np.asarray(inputs["bq"], np.float32)
    wk = np.asarray(inputs["wk"], np.float32); bk = np.asarray(inputs["bk"], np.float32)
    wv = np.asarray(inputs["wv"], np.float32); bv = np.asarray(inputs["bv"], np.float32)
    wo = np.asarray(inputs["wo"], np.float32); bo = np.asarray(inputs["bo"], np.float32)
    g1w = np.asarray(inputs["g1_w"], np.float32); g1b = np.asarray(inputs["g1_b"], np.float32)
    g2w = np.asarray(inputs["g2_w"], np.float32); g2b = np.asarray(inputs["g2_b"], np.float32)
    lng = np.asarray(inputs["ln_g"], np.float32); lnb = np.asarray(inputs["ln_b"], np.float32)

    use_bias_q = bool(np.any(bq)); use_bias_k = bool(np.any(bk))
    row_const = bv @ wo + bo
    use_row_const = bool(np.any(row_const))
    use_ln_aff = bool(np.any(lnb)) or not np.allclose(lng, 1.0)
    cfg = (use_bias_q, use_bias_k, use_row_const, use_ln_aff)
    if cfg not in _CACHE:
        _CACHE[cfg] = _build(cfg)
    nc = _CACHE[cfg]

    bf = ml_dtypes.bfloat16
    xT = np.ascontiguousarray(np.transpose(x, (2, 0, 1)).reshape(D, B * S))
    wv4 = np.zeros((D, 2 * 260), np.float32)
    for h in range(H):
        q2, m = divmod(h, 4)
        wv4[:, 260 * q2 + 65 * m: 260 * q2 + 65 * m + 64] = \
            wv[:, 64 * h:64 * h + 64]
    pos = np.arange(S, dtype=np.float64)

    shared = {
        "xT": xT.astype(bf), "wq8": np.ascontiguousarray(wq * 0.125),
        "wk": wk.astype(bf), "wv4": wv4.astype(bf), "wo": wo, "g1w": g1w,
        "g2w": g2w, "ident": np.eye(128, dtype=np.float32).astype(bf),
        "bq8": np.ascontiguousarray((bq * 0.125).reshape(D, 1)),
        "bk": np.ascontiguousarray(bk.reshape(D, 1)),
        "g1b": np.ascontiguousarray(g1b.reshape(256, 1)),
        "row_const": np.ascontiguousarray(row_const.reshape(1, D)),
        "ln_g_row": np.ascontiguousarray(lng.reshape(1, D)),
        "ln_b_row": np.ascontiguousarray(lnb.reshape(1, D)),
        "g2b": np.asarray(g2b, np.float32).reshape(1, 1),
    }
    in_maps = []
    for c in range(NCORES):
        qbs = [c, 15 - c]
        qidx = np.concatenate([np.arange(QB * qb, QB * qb + QB) for qb in qbs])
        bias_full = np.zeros((2, QB, S), np.float64)
        for qt, qb in enumerate(qbs):
            qv = pos[QB * qb: QB * qb + QB][:, None]
            kv = pos[None, :]
            bias_full[qt] = np.exp(-DECAY * np.abs(qv - kv)) * (qv >= kv)
        biasT = np.zeros((KB, NKB, QPC), np.float64)
        qv = pos[qidx][None, :]
        for kb_i in range(NKB):
            kv = pos[KB * kb_i: KB * kb_i + KB][:, None]
            biasT[:, kb_i, :] = np.exp(-DECAY * np.abs(kv - qv)) * (qv >= kv)
        xq_rows = x[:, qidx, :]
        xrows = np.ascontiguousarray(xq_rows.reshape(2 * B, QB, D))
        xqT = np.ascontiguousarray(
            np.transpose(xq_rows, (2, 0, 1)).reshape(D, B * QPC))
        m = dict(shared)
        m["xrows"] = xrows
        m["xqT"] = xqT
        m["bias_qk"] = bias_full.astype(bf)
        m["biasT"] = biasT.astype(bf)
        in_maps.append(m)

    res = run_bass_kernel_spmd(nc, in_maps, core_ids=list(range(NCORES)))

    attn = np.empty((B, H, S, S), np.float32)
    y = np.empty((B, S, D), np.float32)
    for c in range(NCORES):
        ap = np.asarray(res.results[c]["attn_part"], dtype=np.float32)
        yp = res.results[c]["y_part"]
        for qt, qb in enumerate([c, 15 - c]):
            attn[:, :, QB * qb: QB * qb + QB, :] = ap[:, :, qt]
            y[:, QB * qb: QB * qb + QB, :] = yp[:, qt]
    return y, attn
